# revision 59
# baseline (speedup 1.0000x reference)
"""Trainium2 Bass kernel for GroundwaterModel Jacobi pseudo-timestepping.

Solves 100 Jacobi steps of -div(exp(u) grad p) = f on a [1024,1024] grid,
sharded row-wise (x) across 8 NeuronCores with a 1-row halo exchange per
step (AllGather of pre-weighted boundary rows).

Math: with D = 2*eu + eu_xm + eu_ym (Jacobi diagonal), substitute
q = lam * sqrt(D) * p.  The update becomes

  q'[i,k] = bxu[i,k] q[i+1,k] + bxu[i-1,k] q[i-1,k]
          + by[i,k] q[i,k+1] + by[i,k-1] q[i,k-1] + c[i,k]

with bxu[i,k] = eu[i,j]*rs[i,j]*rs[i+1,j], by[i,k] = eu[i,j]*rs[i,j]*rs[i,j+1],
rs = 1/sqrt(D), c = lam*h^2*f*rs (+ Dirichlet fold at the two y-boundary
columns, Neumann folds at the x edges).  All coefficients are precomputed
on the host in fp64 and shipped as fp16; the iteration state q and the four
shift products run in fp16 on the DVE (2x mode), the partition-dim shifts
and halo injection accumulate in fp32 PSUM via fp16 matmuls (1 cycle/row),
and the per-step inter-core halo is an AllGather of the two boundary
products, issued at the top of each step so it overlaps the interior work.
"""

import numpy as np

GRID = 1024
NCORES = 8
P = 128          # rows per core = SBUF partitions
W = GRID - 2     # computed interior columns j=1..GRID-2
LAM = 1024.0     # q scaling to keep fp16 constants out of the subnormal range
CC_MODE = "ag8"  # "ag8": one 8-core AllGather; "pair": two 2-core AllGathers (hangs NRT)

_cached = {}


def _host_inputs(u, f, n_cores, time_steps, cc_mode=CC_MODE):
    """Per-core input dicts. All per-core variation lives in data."""
    N = u.shape[0]
    h = 1.0 / (N - 1)
    rows = N // n_cores
    Wl = N - 2

    eu = np.exp(u.astype(np.float64))
    eu_xm = np.concatenate([eu[:1, :], eu[:-1, :]], axis=0)
    eu_ym = np.concatenate([eu[:, :1], eu[:, :-1]], axis=1)
    D = 2.0 * eu + eu_xm + eu_ym
    rs = 1.0 / np.sqrt(D)
    h2f = (h * h) * f.astype(np.float64)
    xs = np.arange(N, dtype=np.float64) * h
    bc0 = xs
    bc1 = 1.0 - xs
    j = np.arange(1, N - 1)

    # x-coupling (i,j)<->(i+1,j); row N-1 replaced by the Neumann bottom fold
    bxu = np.zeros((N, Wl))
    bxu[:-1, :] = eu[:-1, j] * rs[:-1, j] * rs[1:, j]
    b_top = eu[0, j] * rs[0, j] * rs[0, j]
    b_bot = eu[N - 1, j] * rs[N - 1, j] * rs[N - 1, j]
    # y-coupling (i,j)<->(i,j+1); column Wl-1 is Dirichlet-folded -> 0
    by = np.zeros((N, Wl))
    by[:, :-1] = eu[:, j[:-1]] * rs[:, j[:-1]] * rs[:, j[:-1] + 1]
    # constants
    c0 = h2f[:, j] * rs[:, j]
    c = c0.copy()
    c[:, 0] += eu_ym[:, 1] * bc0 * rs[:, 1]
    c[:, -1] += eu[:, N - 2] * bc1 * rs[:, N - 2]
    c = LAM * c
    c0 = LAM * c0

    f16 = np.float16
    in_maps = []
    for cidx in range(n_cores):
        r0 = cidx * rows
        A = bxu[r0:r0 + rows].copy()
        if cidx == n_cores - 1:
            A[-1] = b_bot
        Ad = np.zeros((rows, Wl))
        Ad[1:] = bxu[r0:r0 + rows - 1]
        Ad[0] = b_top if cidx == 0 else bxu[r0 - 1]
        By = by[r0:r0 + rows]
        Byd = np.zeros((rows, Wl))
        Byd[:, 1:] = By[:, :-1]

        supT = np.zeros((rows, rows), dtype=f16)
        for i in range(rows - 1):
            supT[i + 1, i] = 1.0          # out[i] += u1[i+1]
        if cidx == 0:
            supT[0, 0] = 1.0              # Neumann top edge via u1[0]
        sdnT = np.zeros((rows, rows), dtype=f16)
        for i in range(1, rows):
            sdnT[i - 1, i] = 1.0          # out[i] += u2[i-1]
        if cidx == n_cores - 1:
            sdnT[rows - 1, rows - 1] = 1.0  # Neumann bottom edge via u2[last]

        if cc_mode == "ag8":
            GR = 2 * n_cores
            eT = np.zeros((GR, rows), dtype=f16)
            if cidx > 0:
                eT[2 * cidx - 1, 0] = 1.0       # prev core's tx2 -> my row 0
            if cidx < n_cores - 1:
                eT[2 * cidx + 2, rows - 1] = 1.0  # next core's tx1 -> my last row
        else:
            # two 2-core AllGathers; gsb rows 0-3 = CC1 pair, 4-7 = CC2 pair,
            # each pair in ascending rank order as [lo_tx1, lo_tx2, hi_tx1, hi_tx2]
            eT = np.zeros((8, rows), dtype=f16)
            if cidx % 2 == 0:
                if cidx + 1 < n_cores:
                    eT[2, rows - 1] = 1.0   # CC1 partner is next: its tx1
                if cidx > 0:
                    eT[4 + 1, 0] = 1.0      # CC2 partner is prev: its tx2
            else:
                eT[1, 0] = 1.0              # CC1 partner is prev: its tx2
                if cidx + 1 < n_cores:
                    eT[4 + 2, rows - 1] = 1.0  # CC2 partner is next: its tx1

        txc = np.zeros((rows, Wl))
        txc[0] = Ad[0]          # tx1: product sent to prev core
        txc[-1] = A[-1]         # tx2: product sent to next core
        # halo-correction coefficients: my row-0 halo total is prev's
        # partial + Ad[0] * (my own top product Ad[0]*q[0] from two steps
        # ago), i.e. corr = Ad[0]^2 * q(t-2); zero on the outer edges
        corrc = np.zeros((rows, Wl))
        if cidx > 0:
            corrc[0] = Ad[0] * Ad[0]
        if cidx < n_cores - 1:
            corrc[-1] = A[-1] * A[-1]
        ident = np.eye(rows, dtype=f16)
        in_maps.append({
            "corrc": corrc.astype(f16), "ident": ident,
            "A": A.astype(f16), "Ad": Ad.astype(f16), "txc": txc.astype(f16),
            "By": By.astype(f16), "Byd": Byd.astype(f16),
            "cp": c[r0:r0 + rows].astype(f16),
            "cp0": c0[r0:r0 + rows].astype(f16),
            "supT": supT, "sdnT": sdnT, "eT": eT,
            "rsl": (rs[r0:r0 + rows, j] / LAM).astype(np.float32),
        })
    return in_maps


def _build(n_cores, time_steps, nx, ny, cc_mode=CC_MODE):
    import concourse.bass as bass
    import concourse.bacc as bacc
    import concourse.mybir as mybir
    from concourse.tile import TileContext

    f32 = mybir.dt.float32
    f16 = mybir.dt.float16
    G = ny
    Wl = G - 2
    rows = nx // n_cores
    GR = 2 * n_cores if cc_mode == "ag8" else 8

    nc = bacc.Bacc(
        "TRN2",
        target_bir_lowering=False,
        debug=False,
        num_devices=n_cores,
    )
    dp = nc.declare_dram_parameter
    A_d = dp("A", [rows, Wl], f16, isOutput=False)
    Ad_d = dp("Ad", [rows, Wl], f16, isOutput=False)
    txc_d = dp("txc", [rows, Wl], f16, isOutput=False)
    corrc_d = dp("corrc", [rows, Wl], f16, isOutput=False)
    ident_d = dp("ident", [rows, rows], f16, isOutput=False)
    By_d = dp("By", [rows, Wl], f16, isOutput=False)
    Byd_d = dp("Byd", [rows, Wl], f16, isOutput=False)
    cp_d = dp("cp", [rows, Wl], f16, isOutput=False)
    cp0_d = dp("cp0", [rows, Wl], f16, isOutput=False)
    supT_d = dp("supT", [rows, rows], f16, isOutput=False)
    sdnT_d = dp("sdnT", [rows, rows], f16, isOutput=False)
    eT_d = dp("eT", [GR, rows], f16, isOutput=False)
    rsl_d = dp("rsl", [rows, Wl], f32, isOutput=False)
    pout_d = dp("pout", [rows, Wl], f32, isOutput=True)

    if cc_mode == "ag8":
        rg_list = [[list(range(n_cores))]]
    else:
        rg_list = [
            [[0, 1], [2, 3], [4, 5], [6, 7]],
            [[0, 7], [1, 2], [3, 4], [5, 6]],
        ]

    with TileContext(nc) as tc:
        with (
            tc.tile_pool(name="coef", bufs=1) as coef,
            tc.tile_pool(name="work", bufs=2) as work,
            tc.tile_pool(name="qp", bufs=2, space="PSUM") as qp,
            tc.tile_pool(name="dramp", bufs=2, space="DRAM") as dramp,
        ):
            # ---- persistent tiles (coefficients + state) ----
            A = coef.tile([rows, Wl], f16, name="A_t")
            Ad = coef.tile([rows, Wl], f16, name="Ad_t")
            txc = coef.tile([rows, Wl], f16, name="txc_t")
            corrc = coef.tile([rows, Wl], f16, name="corrc_t")
            ident = coef.tile([rows, rows], f16, name="ident_t")
            By = coef.tile([rows, Wl], f16, name="By_t")
            Byd = coef.tile([rows, Wl], f16, name="Byd_t")
            cp = coef.tile([rows, Wl], f16, name="cp_t")
            cp0 = coef.tile([rows, Wl], f16, name="cp0_t")
            supT = coef.tile([rows, rows], f16, name="supT_t")
            sdnT = coef.tile([rows, rows], f16, name="sdnT_t")
            eT = coef.tile([GR, rows], f16, name="eT_t")
            rsl = coef.tile([rows, Wl], f32, name="rsl_t")
            # state ring: q(s) lives in qbufs[s % 3]; corr reads q(t-2)
            # while q(t) overwrites q(t-3)'s slot.  Pad cols 0, Wl+1 zero.
            qbufs = [coef.tile([rows, Wl + 2], f16, name=f"qe{i}")
                     for i in range(3)]
            for t_, d_ in ((A, A_d), (Ad, Ad_d), (txc, txc_d),
                           (corrc, corrc_d), (ident, ident_d),
                           (By, By_d), (Byd, Byd_d),
                           (cp, cp_d), (cp0, cp0_d), (supT, supT_d),
                           (sdnT, sdnT_d), (eT, eT_d), (rsl, rsl_d)):
                nc.sync.dma_start(out=t_[:, :], in_=d_[:, :])

            B0 = 512                     # PSUM bank split
            banks = [(0, B0), (B0, Wl)]
            V = nc.vector
            mm = nc.tensor.matmul

            nc.vector.memset(qbufs[0][:, :], 0.0)   # q(0) = 0
            for qb in qbufs[1:]:
                nc.vector.memset(qb[:, 0:1], 0.0)
                nc.vector.memset(qb[:, Wl + 1:Wl + 2], 0.0)

            def send_cc(t, tx):
                # bounce -> AllGather -> gsb; DMAs ride the gpsimd queue
                # (25ns issue vs 565ns on SP)
                bounce = dramp.tile([2, Wl], f16, tag="bounce",
                                    name=f"bounce_{t}")
                nc.gpsimd.dma_start(out=bounce[0:2, :],
                                    in_=tx[0:rows:rows - 1, :])
                gsb = work.tile([GR, Wl], f16, tag="gsb", name=f"gsb_{t}")
                for gi, rg in enumerate(rg_list):
                    gw = 2 * len(rg[0])
                    gkw = {"addr_space": "Shared"} if cc_mode == "ag8" else {}
                    gath = dramp.tile([gw, Wl], f16, tag=f"gath{gi}",
                                      name=f"gath{gi}_{t}", **gkw)
                    nc.gpsimd.collective_compute(
                        "AllGather", mybir.AluOpType.bypass,
                        ins=[bounce.opt()], outs=[gath.opt()],
                        replica_groups=rg,
                    )
                    # gsb load rides SP: on the gpsimd queue it would block
                    # the NEXT step's bounce/trigger behind this collective.
                    # Split per PSUM bank so the first eT matmul can start
                    # once its half lands.
                    for lo, hi in banks:
                        nc.sync.dma_start(
                            out=gsb[gi * gw:(gi + 1) * gw, lo:hi],
                            in_=gath[:, lo:hi])
                return gsb

            # Pipelined exchange: each step sends txc*L computed from local
            # state only (L = y-terms + cp + local x-terms), so step t's
            # AllGather launches without waiting for step t-1's to land.
            # The receiver reconstructs the exact total boundary product:
            #   halo(t-1) = gsb(t-1) partials + corrc * q(t-2)
            # Exact by linearity of the update in the halo term.  The
            # correction rides the local side (L2 = L + corr) so the
            # post-collective chain is just eT matmul + final add.
            gsb = None
            for t in range(1, time_steps + 1):
                if t == 1:
                    q1 = qbufs[1]
                    V.tensor_copy(q1[:, 1:Wl + 1], cp0[:, :])  # q_1 = lam*h2f*rs
                    tx = work.tile([rows, Wl], f16, tag="tx", name="tx_1")
                    V.tensor_mul(tx[:, :], txc[:, :], q1[:, 1:Wl + 1])
                    gsb = send_cc(t, tx)
                    continue

                qpv, qm2, qn = (qbufs[(t - 1) % 3], qbufs[(t - 2) % 3],
                                qbufs[t % 3])
                # products for the x-shifts (PE) and y-shifts (free-dim)
                u1 = work.tile([rows, Wl], f16, tag="u1", name=f"u1_{t}")
                u2 = work.tile([rows, Wl], f16, tag="u2", name=f"u2_{t}")
                y1 = work.tile([rows, Wl], f16, tag="y1", name=f"y1_{t}")
                y2 = work.tile([rows, Wl], f16, tag="y2", name=f"y2_{t}")
                V.tensor_mul(u1[:, :], Ad[:, :], qpv[:, 1:Wl + 1])
                V.tensor_mul(u2[:, :], A[:, :], qpv[:, 1:Wl + 1])
                V.tensor_mul(y1[:, :], By[:, :], qpv[:, 2:Wl + 2])
                V.tensor_mul(y2[:, :], Byd[:, :], qpv[:, 0:Wl])
                # PE group 1: local x-terms + cp
                psx = qp.tile([rows, Wl], f32, tag="psx", name=f"psx_{t}")
                for lo, hi in banks:
                    mm(psx[:, lo:hi], supT[:, :], u1[:, lo:hi],
                       start=True, stop=False)
                for lo, hi in banks:
                    mm(psx[:, lo:hi], sdnT[:, :], u2[:, lo:hi],
                       start=False, stop=False)
                for lo, hi in banks:
                    mm(psx[:, lo:hi], ident[:, :], cp[:, lo:hi],
                       start=False, stop=True)
                a1 = work.tile([rows, Wl], f16, tag="a1", name=f"a1_{t}")
                V.tensor_add(a1[:, :], y1[:, :], y2[:, :])
                # L: this step's update minus the halo term (local only)
                L = work.tile([rows, Wl], f16, tag="L", name=f"L_{t}")
                V.tensor_add(L[:, :], a1[:, :], psx[:, :])
                if t < time_steps:
                    # launch this step's exchange from L -- does not wait on
                    # the previous collective
                    tx = work.tile([rows, Wl], f16, tag="tx", name=f"tx_{t}")
                    V.tensor_mul(tx[:, :], txc[:, :], L[:, :])
                    gsb_next = send_cc(t, tx)
                else:
                    gsb_next = None
                # local side of the halo: L2 = L + corrc*q(t-2), computed
                # while the collective is in flight
                corr = work.tile([rows, Wl], f16, tag="corr",
                                 name=f"corr_{t}")
                V.tensor_mul(corr[:, :], corrc[:, :], qm2[:, 1:Wl + 1])
                L2 = work.tile([rows, Wl], f16, tag="L2", name=f"L2_{t}")
                V.tensor_add(L2[:, :], L[:, :], corr[:, :])
                # post-collective chain, pipelined per PSUM bank:
                # eT partial-product injection, then the final add
                psh = qp.tile([rows, Wl], f32, tag="psh", name=f"psh_{t}")
                for lo, hi in banks:
                    mm(psh[:, lo:hi], eT[:, :], gsb[:, lo:hi],
                       start=True, stop=True)
                    V.tensor_add(qn[:, 1 + lo:1 + hi], L2[:, lo:hi],
                                 psh[:, lo:hi])
                gsb = gsb_next

            qfin = qbufs[time_steps % 3]
            out_sb = coef.tile([rows, Wl], f32, name="out_sb")
            nc.vector.tensor_mul(out_sb[:, :], qfin[:, 1:Wl + 1], rsl[:, :])
            nc.sync.dma_start(out=pout_d[:, :], in_=out_sb[:, :])

    nc.finalize()
    return nc


def _get_nc(n_cores, time_steps, nx, ny):
    key = (n_cores, time_steps, nx, ny)
    if key not in _cached:
        _cached[key] = _build(n_cores, time_steps, nx, ny)
    return _cached[key]


def kernel(u, f, time_steps):
    from concourse.bass_utils import run_bass_kernel_spmd

    u = np.asarray(u)
    f = np.asarray(f)
    ts = int(time_steps)
    N = u.shape[0]
    n_cores = NCORES
    nc = _get_nc(n_cores, ts, N, u.shape[1])
    in_maps = _host_inputs(u, f, n_cores, ts)
    res = run_bass_kernel_spmd(nc, in_maps, list(range(n_cores))).results
    interior = np.concatenate([r["pout"] for r in res], axis=0)
    h = 1.0 / (N - 1)
    xs = (np.arange(N, dtype=np.float64) * h).astype(np.float32)
    out = np.empty((N, N), dtype=np.float32)
    out[:, 1:N - 1] = interior
    out[:, 0] = xs
    out[:, N - 1] = 1.0 - xs
    return out
